# revision 51
# baseline (speedup 1.0000x reference)
"""Trainium2 Bass kernel for nn_CapsuleLayer — channel-sharded v6.

v5 -> v6: latency-focused restructure guided by CoreSim timelines
(~71.4us -> ~66.5us slope-measured on HW).
- s staging split per batch-half: PSUM->SBUF bf16 copy (ACT for h0, DVE
  for h1) + DRAM DMA of half h overlap the other half's matmuls, so the
  AllReduce triggers earlier.
- collective return split into two half DMAs; squash runs per half so
  the first half's activations start one DMA earlier.  |s|^2 per unit
  comes from one DVE pass (cumsum(s*s) + stride-S diffs) instead of the
  ACT Square + TensorReduce hop pair.
- g-matmul runs batch-half-outer (all h0 chunks, then h1) into three
  per-bank PSUM tiles (separate tiles kill false WAR serialization);
  the h0 pass overlaps the second squash half, and MULSCAN + segment
  diffs + per-bank selx matmuls pipeline behind the h1 pass.  One PSUM
  accumulation group per bank: start only on the bank's first write
  (start=True lazily zeroes the whole 2KB bank), stop on its last.
- the b-logit update accumulates in PSUM: an identity-matmul preloads
  b_old into the bank, the selx matmuls accumulate the new agreement,
  and Exp reads PSUM directly (kills the serial tensor_add).
- weff computed in 3 chunk-groups split across DVE/gpsimd so the next
  s-matmul starts on group 0 while later groups multiply.
- input loads spread across the DMA queues (x_t on SP, w on ACT,
  x_b/selectors via gpsimd SWDGE) to shorten the single-shot lead-in
  and keep iteration-0 staging off a busy queue.
- final AllToAll tail runs per half: return DMA -> sel16 partition-sum
  matmul -> copy -> squash -> out DMA, h0 one stage ahead of h1.

Measured dead ends (this topology, axon-tunneled 8-core group):
AllGather+local-sum (+14us), two pipelined half-ARs (+14us: collectives
serialize, never pipeline), PE-warming dummies (+7us), 4-rank replica
groups (unsupported: mesh/Shared need >4 ranks), fp8 DoubleRow (s rel
err ~3% — quantization error does not average down), dropping a routing
iteration (19.5% error).

Sharding: C split 8 ways (144 channels / core), K = 1152 -> 9 chunks of 128
with partition p = (c%16)*8 + i.  Per iteration the s_j partial is
reduced across cores in bf16 (AllReduce); squash is replicated (iters
0-1).  The final iteration uses AllToAll + a selection-matmul partition
reduce, and each core squashes + outputs only its own 32-batch slice.
Routing logits are channel-local, held replicated over the 8
i-partitions.
"""

import numpy as np

import concourse.bass as bass
import concourse.bacc as bacc
import concourse.tile as tile
from concourse import mybir
from concourse import bass_utils

# ------------------------------------------- custom DVE op: prefix(W*G)


def _register_mulscan():
    import numpy as np
    from concourse import dve_ops
    from concourse.dve_spec import Spec, Src0, Src1, AluOp, scan, lower
    from concourse.dve_uop import DveOpSpec

    name = "CAPS_MULSCAN_ANT"
    for op in dve_ops.OPS:
        if op.name == name:
            return op
    spec = Spec(
        body=scan(AluOp.ADD, Src0 * Src1),
        reference=lambda in0, in1, s0, s1, imm2: np.cumsum(
            np.asarray(in0, np.float32).reshape(in0.shape[0], -1)
            * np.asarray(in1, np.float32).reshape(in1.shape[0], -1),
            axis=1,
        ),
    )
    row = dve_ops._CUSTOM_DVE_ROW_BASE + len(dve_ops.OPS)
    shas = {}
    for ver in ("v3", "v4"):
        s = DveOpSpec(name=name, opcode=row, uops=lower(spec, ver=ver), rd1_en=True)
        shas[ver] = s.sha(ver)
    op = dve_ops.DveOp(name, spec, subdim=False, uops_sha=shas)
    dve_ops.OPS.append(op)
    dve_ops.CUSTOM_DVE_SPECS[name] = spec
    dve_ops._SUB_OPCODE_FOR_NAME[name] = row
    return op


MULSCAN = _register_mulscan()


def _pin_act_tables():
    """Make natural_log_exp_and_others the unique candidate set for
    exp/ln/square so bacc's table-load pass never alternates sets."""
    import functools
    import concourse.bacc as _bacc
    import concourse.hw_specs as _hw
    if getattr(_bacc, "_caps_act_pinned", False):
        return
    orig = _hw.get_activation_tables

    @functools.cache
    def pinned(module_arch):
        tables = dict(orig(module_arch))
        keep = "natural_log_exp_and_others"
        assert keep in tables
        only = tables[keep]
        excl = {f for f in only}
        out = {}
        for name, funcs in tables.items():
            if name == keep:
                out[name] = funcs
            else:
                out[name] = funcs - excl
        return out

    _bacc.get_activation_tables = pinned
    _hw.get_activation_tables_orig = orig
    _bacc._caps_act_pinned = True


_pin_act_tables()

# ---------------------------------------------------------------- constants
B, I, C, U, S = 256, 8, 1152, 10, 16
NCORES = 8
CL = C // NCORES            # 144 channels per core
KT = CL * I // 128          # 9 K-chunks of 128 (16 c x 8 i)
NUS = U * S                 # 160
NSEG = KT * U               # 90 (chunk, u) segments
NSEG_J = 3 * U              # 30 segments per PSUM bank (3 chunks)
EPS = 1e-8
NUM_ROUTING = 3

MM_CFG = "bf16"

_DT = {
    "f32": mybir.dt.float32,
    "bf16": mybir.dt.bfloat16,
}


def _np_dt(cfg):
    if cfg == "bf16":
        import ml_dtypes
        return ml_dtypes.bfloat16
    return np.float32


# ---------------------------------------------------------------- device code
def build_nc(cfg=MM_CFG, repeat=1, collectives=True, final_rs=True, solo=False,
             warm_pe=0, sq_legacy=False, g_legacy=False, agmode=False,
             arsplit=False):
    nc = bacc.Bacc(
        "TRN2",
        target_bir_lowering=False,
        debug=False,
        num_devices=1 if collectives is False else NCORES,
    )
    mdt = _DT[cfg]
    f32 = mybir.dt.float32

    w_d = nc.dram_tensor("w_sb", [128, KT * NUS], mdt, kind="ExternalInput")
    xt_d = nc.dram_tensor("x_t", [128, KT * B], mdt, kind="ExternalInput")
    xb_d = nc.dram_tensor("x_b", [128, 2 * KT * 128], mdt, kind="ExternalInput")
    selx_d = nc.dram_tensor("selx", [128, 128], f32, kind="ExternalInput")
    ident_d = nc.dram_tensor("ident", [128, 128], f32, kind="ExternalInput")
    sel16_d = nc.dram_tensor("sel16", [128, 16], mdt, kind="ExternalInput")
    out_d = nc.dram_tensor("v_out", [16, 2 * NUS], f32, kind="ExternalOutput")

    with tile.TileContext(nc) as tc:
        with (
            tc.tile_pool(name="singles", bufs=1) as singles,
            tc.tile_pool(name="work", bufs=2) as work,
            tc.tile_pool(name="small", bufs=2) as small,
            tc.tile_pool(name="ps_s", bufs=1, space="PSUM") as ps_s,
            tc.tile_pool(name="ps_g", bufs=1, space="PSUM") as ps_g,
            tc.tile_pool(name="ps_b", bufs=1, space="PSUM") as ps_b,
            tc.tile_pool(name="ps_w", bufs=1, space="PSUM") as ps_w,
            tc.tile_pool(name="dram", bufs=2, space="DRAM") as dram,
        ):
            # ---------------- persistent SBUF loads, spread across queues:
            # x_t (3 chunks) then x_b on SP; w (2 chunks) on DVE's queue;
            # selectors on ACT's queue (needed later than x_t/w).
            x_t = singles.tile([128, KT, B], mdt)
            w_sb = singles.tile([128, KT, U, S], mdt)
            for r in range(3):
                nc.sync.dma_start(
                    out=x_t[:, r * 3:(r + 1) * 3, :],
                    in_=xt_d[:, r * 3 * B:(r + 1) * 3 * B],
                )
            nc.scalar.dma_start(
                out=w_sb[:, :6, :, :], in_=w_d[:, :6 * NUS],
            )
            nc.scalar.dma_start(
                out=w_sb[:, 6:, :, :], in_=w_d[:, 6 * NUS:],
            )
            x_b = singles.tile([128, 2, KT, 128], mdt)
            nc.gpsimd.dma_start(out=x_b[:], in_=xb_d[:])
            selx = singles.tile([128, 128], f32)
            nc.gpsimd.dma_start(out=selx[:], in_=selx_d[:])
            ident = singles.tile([128, 128], f32)
            nc.gpsimd.dma_start(out=ident[:], in_=ident_d[:])
            sel16 = singles.tile([128, 16], mdt)
            nc.gpsimd.dma_start(out=sel16[:], in_=sel16_d[:])
            eps_sb = singles.tile([128, 1], f32)
            nc.vector.memset(eps_sb[:], EPS)

            def pe_warmers(n):
                """Dummy matmuls issued behind a collective trigger: they
                drain from the PE queue while the collective is in flight,
                keeping the HAM activity window busy so the clock gate stays
                at full rate for the real matmuls that follow."""
                if not n:
                    return
                wscr = ps_w.tile([128, 512], f32, tag="wscr")
                for _ in range(n):
                    nc.tensor.matmul(
                        out=wscr[:, :B], lhsT=x_t[:, 0, :128],
                        rhs=x_t[:, 0, :], start=True, stop=True,
                    )

            def squash_half(s_ap, alpha, vdt, P, tg, v_out=None,
                            legacy=False):
                """v = squash(alpha * s) for one batch half; s_ap [P, U, S].

                |s|^2 per u comes from one DVE pass: cumsum(s*s) then
                stride-S differences — no ACT Square / TensorReduce hops.
                """
                sq = small.tile([P, U], f32, tag=f"sq{tg}")
                if sq_legacy or legacy:
                    s2 = small.tile([P, U, S], f32, tag=f"s2{tg}")
                    nc.scalar.activation(
                        out=s2[:], in_=s_ap,
                        func=mybir.ActivationFunctionType.Square,
                    )
                    nc.vector.reduce_sum(
                        out=sq[:], in_=s2[:], axis=mybir.AxisListType.X
                    )
                else:
                    s_flat = s_ap.rearrange("p u s -> p (u s)")
                    prefs = small.tile([P, (U + 1) * S], f32, tag=f"ps{tg}")
                    nc.vector.memset(prefs[:, 0:1], 0.0)
                    nc.vector._custom_dve(
                        MULSCAN, out=prefs[:, 1:1 + U * S],
                        in0=s_flat, in1=s_flat,
                    )
                    ends = prefs[:, S:S + U * S].rearrange(
                        "p (n s) -> p n s", s=S
                    )[:, :, 0]
                    prevs = prefs[:, 0:U * S].rearrange(
                        "p (n s) -> p n s", s=S
                    )[:, :, 0]
                    nc.vector.scalar_tensor_tensor(
                        out=sq[:], in0=prevs, scalar=-1.0, in1=ends,
                        op0=mybir.AluOpType.mult, op1=mybir.AluOpType.add,
                    )
                if alpha != 1.0:
                    t = small.tile([P, U], f32, tag=f"t{tg}")
                    nc.vector.tensor_scalar_mul(t[:], sq[:], alpha * alpha)
                else:
                    t = sq
                lnt = small.tile([P, U], f32, tag=f"lnt{tg}")
                nc.scalar.activation(
                    out=lnt[:], in_=t[:],
                    func=mybir.ActivationFunctionType.Ln, bias=eps_sb[:P, :],
                )
                rt = small.tile([P, U], f32, tag=f"rt{tg}")
                nc.scalar.activation(
                    out=rt[:], in_=lnt[:],
                    func=mybir.ActivationFunctionType.Exp, scale=0.5,
                )
                dd = small.tile([P, U], f32, tag=f"dd{tg}")
                nc.vector.scalar_tensor_tensor(
                    out=dd[:], in0=t[:], scalar=1.0, in1=rt[:],
                    op0=mybir.AluOpType.add, op1=mybir.AluOpType.mult,
                )
                g = small.tile([P, U], f32, tag=f"g{tg}")
                nc.vector.reciprocal(g[:], dd[:])
                af = small.tile([P, U], f32, tag=f"af{tg}")
                nc.vector.scalar_tensor_tensor(
                    out=af[:], in0=t[:], scalar=float(alpha), in1=g[:],
                    op0=mybir.AluOpType.mult, op1=mybir.AluOpType.mult,
                )
                if v_out is None:
                    v = small.tile([P, U, S], vdt, tag=f"v{tg}")
                    v_ap = v[:]
                else:
                    v, v_ap = None, v_out
                nc.vector.tensor_mul(
                    v_ap, s_ap,
                    af[:, :, None].broadcast_to([P, U, S]),
                )
                return v

            # ------------------------------------------------ routing loop
            for _rep in range(repeat):
                c_sm = None         # [128, KT, U] f32 softmax'd coupling
                b_sb = None         # [128, NSEG] f32 logits (i-replicated)
                for it in range(NUM_ROUTING):
                    alpha = 1.0 / U if it == 0 else 1.0
                    # ------------ weff = W * c in 3 chunk-groups (skip it 0)
                    if it == 0:
                        weff = w_sb
                    else:
                        weff = work.tile([128, KT, U, S], mdt, tag="weff")
                        for gj, eng in ((0, nc.gpsimd), (1, nc.gpsimd),
                                        (2, nc.vector)):
                            sl = slice(3 * gj, 3 * gj + 3)
                            eng.tensor_mul(
                                weff[:, sl],
                                w_sb[:, sl],
                                c_sm[:, sl, :, None].broadcast_to(
                                    [128, 3, U, S]
                                ),
                            )
                    weff_flat = weff[:].rearrange("p k u s -> p (k u s)")
                    last = it == NUM_ROUTING - 1
                    # ------------ s partial over 2 b-halves, staged per half
                    # (separate PSUM tiles per half so the h0 drain never
                    # serializes against the h1 matmuls)
                    use_split = arsplit and collectives is True and not last
                    s_stage = work.tile([128, 2, NUS], mdt, tag=f"s_stage{it}")
                    if use_split:
                        ar_ins = [
                            dram.tile([128, NUS], mdt, tag=f"ar_in{it}h{h}",
                                      name=f"ar_in_h{h}")
                            for h in range(2)
                        ]
                    else:
                        ar_in = dram.tile([128, 2, NUS], mdt, tag=f"ar_in{it}")
                    # chunk-group-interleaved: both halves consume each
                    # weff bank as it lands, so the halves finish ~one bank
                    # apart and the collective triggers earlier than with a
                    # full second-half pass at the end
                    s_pss = [
                        ps_s.tile([128, 512], f32, tag=f"s{h}",
                                  name=f"s_ps{h}")
                        for h in range(2)
                    ]
                    for grp in range(3):
                        for h in range(2):
                            for kb in range(3 * grp, 3 * grp + 3):
                                nc.tensor.matmul(
                                    out=s_pss[h][:, :NUS],
                                    lhsT=x_t[:, kb, h * 128:(h + 1) * 128],
                                    rhs=weff_flat[
                                        :, kb * NUS:(kb + 1) * NUS
                                    ],
                                    start=(kb == 0),
                                    stop=(kb == KT - 1),
                                    skip_group_check=True,
                                )
                    for h in range(2):
                        if h == 0:
                            nc.scalar.copy(
                                out=s_stage[:, h], in_=s_pss[h][:, :NUS]
                            )
                        else:
                            nc.vector.tensor_copy(
                                s_stage[:, h], s_pss[h][:, :NUS]
                            )
                        nc.sync.dma_start(
                            out=ar_ins[h][:] if use_split else ar_in[:, h],
                            in_=s_stage[:, h],
                        )
                    if not use_split:
                        ar_flat = ar_in[:].rearrange("p h m -> p (h m)")
                    if last:
                        # ------------ final: AllToAll; each core reduces +
                        # squashes only its own 32 batches
                        ar_out = dram.tile([128, 2 * NUS], mdt, tag="ar_outf")
                        if collectives is True:
                            nc.gpsimd.collective_compute(
                                "AllToAll",
                                mybir.AluOpType.bypass,
                                replica_groups=[[i] for i in range(NCORES)] if solo else [list(range(NCORES))],
                                ins=[ar_flat.opt()],
                                outs=[ar_out[:].opt()],
                            )
                        else:
                            nc.sync.dma_start(out=ar_out[:], in_=ar_flat)
                        pe_warmers(warm_pe)
                        # per-half tail: return DMA -> partition-sum matmul
                        # -> copy -> squash -> out DMA, h0 one step ahead
                        a2a_sb = work.tile([128, 2, NUS], mdt, tag="a2a_sb")
                        # reuse the g bank-0 buffer: the g section never runs
                        # in the final iteration, and ps_b now holds the three
                        # per-bank logit tiles (PSUM budget stays at 8 banks)
                        s16_scr = ps_g.tile([128, 512], f32, tag="g0",
                                            name="s16_scr")
                        s16_ps = s16_scr[:16, :2 * NUS]
                        s16_sb = work.tile([16, 2, U, S], f32, tag="s16_sb")
                        v_fin = work.tile([16, 2, U, S], f32, tag="v_fin")
                        for h in range(2):
                            nc.sync.dma_start(
                                out=a2a_sb[:, h],
                                in_=ar_out[:, h * NUS:(h + 1) * NUS],
                            )
                            nc.tensor.matmul(
                                out=s16_ps[:, h * NUS:(h + 1) * NUS],
                                lhsT=sel16[:], rhs=a2a_sb[:, h],
                                start=(h == 0), stop=(h == 1),
                                skip_group_check=True,
                            )
                            nc.scalar.copy(
                                out=s16_sb[:, h],
                                in_=s16_ps[
                                    :, h * NUS:(h + 1) * NUS
                                ].rearrange("p (u s) -> p u s", u=U),
                            )
                            squash_half(
                                s16_sb[:, h], alpha, f32, 16, f"f{h}",
                                v_out=v_fin[:, h],
                            )
                            nc.sync.dma_start(
                                out=out_d[:, h * NUS:(h + 1) * NUS],
                                in_=v_fin[:, h].rearrange(
                                    "p u s -> p (u s)"
                                ),
                            )
                        break
                    # ------------ reduce across cores (bf16): AllReduce, or
                    # AllGather + local tree-sum split across DVE/gpsimd
                    groups = [[i] for i in range(NCORES)] if solo else [list(range(NCORES))]
                    s_sb = work.tile([128, 2, U, S], mdt, tag="s_sb")
                    v2 = work.tile([128, 2, U, S], mdt, tag="v2")
                    if agmode and collectives is True:
                        # AG concatenates the ranks' raveled buffers: rank-
                        # major [r, p, m] layout in DRAM.
                        ag_out = dram.tile(
                            [NCORES, 128, 2 * NUS], mdt,
                            tag=f"ag_out{it}", addr_space="Shared",
                        )
                        nc.gpsimd.collective_compute(
                            "AllGather",
                            mybir.AluOpType.bypass,
                            replica_groups=groups,
                            ins=[ar_flat.opt()],
                            outs=[ag_out[:].rearrange("r p m -> (r p) m").opt()],
                        )
                        gsb = work.tile([128, NCORES, 2 * NUS], mdt,
                                        tag=f"gsb{it}")
                        nc.sync.dma_start(
                            out=gsb[:, :4],
                            in_=ag_out[:4].rearrange("r p m -> p r m"),
                        )
                        nc.scalar.dma_start(
                            out=gsb[:, 4:],
                            in_=ag_out[4:].rearrange("r p m -> p r m"),
                        )
                        q = work.tile([128, 4, 2 * NUS], mdt, tag=f"q{it}")
                        nc.vector.tensor_add(q[:, 0], gsb[:, 0], gsb[:, 1])
                        nc.vector.tensor_add(q[:, 1], gsb[:, 2], gsb[:, 3])
                        nc.gpsimd.tensor_add(q[:, 2], gsb[:, 4], gsb[:, 5])
                        nc.gpsimd.tensor_add(q[:, 3], gsb[:, 6], gsb[:, 7])
                        r2 = work.tile([128, 2, 2 * NUS], mdt, tag=f"r2{it}")
                        nc.vector.tensor_add(r2[:, 0], q[:, 0], q[:, 1])
                        nc.gpsimd.tensor_add(r2[:, 1], q[:, 2], q[:, 3])
                        nc.vector.tensor_add(
                            s_sb[:].rearrange("p h u s -> p (h u s)"),
                            r2[:, 0], r2[:, 1],
                        )
                        for h in range(2):
                            squash_half(s_sb[:, h], alpha, mdt, 128,
                                        f"r{h}", v_out=v2[:, h])
                        vs = [v2[:, 0], v2[:, 1]]
                    elif arsplit and collectives is True:
                        # two pipelined half-batch AllReduces: h0 launches
                        # ~1.3us before h1; tests whether back-to-back
                        # collectives overlap in the firmware pipeline
                        vs = [v2[:, 0], v2[:, 1]]
                        for h in range(2):
                            ar_out_h = dram.tile(
                                [128, NUS], mdt,
                                tag=f"ar_out{it}h{h}", addr_space="Shared",
                            )
                            nc.gpsimd.collective_compute(
                                "AllReduce",
                                mybir.AluOpType.add,
                                replica_groups=groups,
                                ins=[ar_ins[h][:].opt()],
                                outs=[ar_out_h[:].opt()],
                            )
                            nc.sync.dma_start(
                                out=s_sb[:, h], in_=ar_out_h[:],
                            )
                            squash_half(s_sb[:, h], alpha, mdt, 128,
                                        f"r{h}", v_out=v2[:, h])
                    else:
                        ar_out = dram.tile([128, 2 * NUS], mdt, tag=f"ar_out{it}", addr_space="Shared")
                        if collectives is True:
                            nc.gpsimd.collective_compute(
                                "AllReduce",
                                mybir.AluOpType.add,
                                replica_groups=groups,
                                ins=[ar_flat.opt()],
                                outs=[ar_out[:].opt()],
                            )
                        else:
                            nc.sync.dma_start(out=ar_out[:], in_=ar_flat)
                        pe_warmers(warm_pe)
                        vs = [v2[:, 0], v2[:, 1]]
                        for h in range(2):
                            nc.sync.dma_start(
                                out=s_sb[:, h],
                                in_=ar_out[:, h * NUS:(h + 1) * NUS],
                            )
                            squash_half(s_sb[:, h], alpha, mdt, 128,
                                        f"r{h}", v_out=v2[:, h])
                    # ------------ g matmul, batch-half outer; MULSCAN + segment
                    # diffs pipelined per PSUM bank behind the h1 pass
                    # (separate PSUM tile per bank to avoid false deps)
                    g_banks = [
                        ps_g.tile([128, 512], f32, tag=f"g{j}",
                                  name=f"g_bank{j}")
                        for j in range(3)
                    ]
                    pref = work.tile([128, 3, 512], f32, tag="pref")
                    nc.vector.memset(pref[:, :, 0:1], 0.0)
                    d = small.tile([128, NSEG], f32, tag="d")
                    b_banks = [
                        ps_b.tile([128, NSEG_J], f32, tag=f"b{j}",
                                  name=f"b_bank{j}")
                        for j in range(3)
                    ]
                    if b_sb is not None:
                        # preload each b bank with the running logits so the
                        # selx matmuls accumulate the update in PSUM (kills
                        # the serial tensor_add on the critical path)
                        for j in range(3):
                            nc.tensor.matmul(
                                out=b_banks[j][:], lhsT=ident[:],
                                rhs=b_sb[:, j * NSEG_J:(j + 1) * NSEG_J],
                                start=True, stop=False,
                                skip_group_check=True,
                            )
                    if g_legacy:
                        for kb in range(KT):
                            for h in range(2):
                                nc.tensor.matmul(
                                    out=g_banks[kb // 3][
                                        :,
                                        (kb % 3) * NUS:(kb % 3) * NUS + NUS,
                                    ],
                                    lhsT=x_b[:, h, kb, :],
                                    rhs=vs[h].rearrange(
                                        "p u s -> p (u s)"
                                    ),
                                    start=(h == 0),
                                    stop=(h == 1),
                                )
                            if kb % 3 == 2:
                                j = kb // 3
                                nc.vector._custom_dve(
                                    MULSCAN,
                                    out=pref[:, j, 1:1 + 3 * NUS],
                                    in0=w_sb[:, 3 * j:3 * j + 3].rearrange(
                                        "p k u s -> p (k u s)"
                                    ),
                                    in1=g_banks[j][:, :3 * NUS],
                                )
                                ends = pref[
                                    :, j, S:S + NSEG_J * S
                                ].rearrange("p (n s) -> p n s", s=S)[:, :, 0]
                                prevs = pref[
                                    :, j, 0:NSEG_J * S
                                ].rearrange("p (n s) -> p n s", s=S)[:, :, 0]
                                nc.vector.scalar_tensor_tensor(
                                    out=d[:, j * NSEG_J:(j + 1) * NSEG_J],
                                    in0=prevs, scalar=-1.0, in1=ends,
                                    op0=mybir.AluOpType.mult,
                                    op1=mybir.AluOpType.add,
                                )
                                nc.tensor.matmul(
                                    out=b_banks[j][:],
                                    lhsT=selx[:],
                                    rhs=d[:, j * NSEG_J:(j + 1) * NSEG_J],
                                    start=(b_sb is None),
                                    stop=True,
                                    skip_group_check=True,
                                )
                    else:
                        pass
                    for h in ([] if g_legacy else range(2)):
                        vh = vs[h][:].rearrange("p u s -> p (u s)")
                        for kb in range(KT):
                            # One accumulation group per PSUM bank: start
                            # lazily zeroes the whole 2KB bank, so only the
                            # bank's first write may set it; the other h0
                            # writes land on pending-zero bytes (= overwrite).
                            nc.tensor.matmul(
                                out=g_banks[kb // 3][
                                    :, (kb % 3) * NUS:(kb % 3) * NUS + NUS,
                                ],
                                lhsT=x_b[:, h, kb, :],
                                rhs=vh,
                                start=(h == 0 and kb % 3 == 0),
                                stop=(h == 1 and kb % 3 == 2),
                                skip_group_check=True,
                            )
                            if h == 1 and kb % 3 == 2:
                                j = kb // 3
                                nc.vector._custom_dve(
                                    MULSCAN,
                                    out=pref[:, j, 1:1 + 3 * NUS],
                                    in0=w_sb[:, 3 * j:3 * j + 3].rearrange(
                                        "p k u s -> p (k u s)"
                                    ),
                                    in1=g_banks[j][:, :3 * NUS],
                                )
                                ends = pref[
                                    :, j, S:S + NSEG_J * S
                                ].rearrange("p (n s) -> p n s", s=S)[:, :, 0]
                                prevs = pref[
                                    :, j, 0:NSEG_J * S
                                ].rearrange("p (n s) -> p n s", s=S)[:, :, 0]
                                nc.vector.scalar_tensor_tensor(
                                    out=d[:, j * NSEG_J:(j + 1) * NSEG_J],
                                    in0=prevs, scalar=-1.0, in1=ends,
                                    op0=mybir.AluOpType.mult,
                                    op1=mybir.AluOpType.add,
                                )
                                # group-sum over the 8 i-partitions for this
                                # bank, replicated back to all 128 partitions
                                # (selx[p,q] = (p//8==q//8)/B)
                                nc.tensor.matmul(
                                    out=b_banks[j][:],
                                    lhsT=selx[:],
                                    rhs=d[:, j * NSEG_J:(j + 1) * NSEG_J],
                                    start=(b_sb is None),
                                    stop=True,
                                    skip_group_check=True,
                                )
                    # ------------ per-bank softmax over u (independent
                    # per chunk) + weff + next-iteration s-matmuls: bank j's
                    # chain starts as soon as its selx matmul lands, so the
                    # first chunks' coupling weights and matmuls overlap the
                    # later banks' MULSCANs.  The persistence copies (only
                    # needed for the next non-final iteration) run off the
                    # critical path.
                    if it < NUM_ROUTING - 2:
                        b_new = small.tile([128, NSEG], f32, tag=f"bn{it}")
                        for j in range(3):
                            nc.scalar.copy(
                                out=b_new[:, j * NSEG_J:(j + 1) * NSEG_J],
                                in_=b_banks[j][:],
                            )
                        b_sb = b_new
                    else:
                        b_sb = None
                    c_sm = small.tile([128, KT, U], f32, tag="c_sm")
                    for j in range(3):
                        e = small.tile([128, 3, U], f32, tag=f"e{j}")
                        nc.scalar.activation(
                            out=e[:],
                            in_=b_banks[j][:].rearrange(
                                "p (k u) -> p k u", u=U
                            ),
                            func=mybir.ActivationFunctionType.Exp,
                        )
                        se = small.tile([128, 3], f32, tag=f"se{j}")
                        nc.vector.reduce_sum(
                            out=se[:], in_=e[:], axis=mybir.AxisListType.X
                        )
                        re = small.tile([128, 3], f32, tag=f"re{j}")
                        nc.vector.reciprocal(re[:], se[:])
                        nc.vector.tensor_mul(
                            c_sm[:, 3 * j:3 * j + 3], e[:],
                            re[:, :, None].broadcast_to([128, 3, U]),
                        )

    nc.compile()
    return nc


# ---------------------------------------------------------------- host prep
def prep_inputs(x, weight, cfg=MM_CFG):
    """Full inputs -> per-core in_maps with kernel-ready layouts."""
    x = np.asarray(x, dtype=np.float32)
    weight = np.asarray(weight, dtype=np.float32)
    npdt = _np_dt(cfg)

    selx = np.zeros((128, 128), np.float32)
    pp = np.arange(128)
    selx[:, :] = (pp[:, None] // 8 == pp[None, :] // 8) / B
    sel16 = (pp[:, None] % 16 == np.arange(16)[None, :]).astype(npdt)

    in_maps = []
    for k in range(NCORES):
        cs = slice(k * CL, (k + 1) * CL)
        w = (
            weight[cs]
            .reshape(KT, 16, U, S, I)
            .transpose(1, 4, 0, 2, 3)          # [16, I, KT, U, S]
            .reshape(128, KT * U * S)
        )
        xs = x[:, :, cs]                        # [B, I, CL]
        x_t = (
            xs.transpose(2, 1, 0)               # [CL, I, B]
            .reshape(KT, 16, I, B)
            .transpose(1, 2, 0, 3)              # [16, I, KT, B]
            .reshape(128, KT * B)
        )
        x_b = (
            xs.transpose(0, 2, 1)               # [B, CL, I]
            .reshape(2, 128, KT, 16 * I)
            .transpose(1, 0, 2, 3)
            .reshape(128, 2 * KT * 128)
        )
        in_maps.append({
            "w_sb": np.ascontiguousarray(w, dtype=npdt),
            "x_t": np.ascontiguousarray(x_t, dtype=npdt),
            "x_b": np.ascontiguousarray(x_b, dtype=npdt),
            "selx": selx,
            "ident": np.eye(128, dtype=np.float32),
            "sel16": sel16,
        })
    return in_maps


def assemble_output(results):
    # AllToAll leaves rank r with batches {h*128 + 16r + p}
    out = np.empty((B, U, S, 1), np.float32)
    for r in range(NCORES):
        v = results[r]["v_out"].astype(np.float32).reshape(16, 2, U, S)
        for h in range(2):
            out[h * 128 + 16 * r:h * 128 + 16 * r + 16] = v[:, h][..., None]
    return out


_NC_CACHE = {}


def _get_nc(cfg=MM_CFG):
    if cfg not in _NC_CACHE:
        _NC_CACHE[cfg] = build_nc(cfg)
    return _NC_CACHE[cfg]


def kernel(x, weight):
    nc = _get_nc()
    in_maps = prep_inputs(x, weight)
    res = bass_utils.run_bass_kernel_spmd(
        nc, in_maps, core_ids=list(range(NCORES))
    )
    return assemble_output(res.results)
